# revision 8
# baseline (speedup 1.0000x reference)
"""DKVMN attention-guided memory kernel for Trainium2, 8-core data-parallel.

Layout/algorithm:
  - Shard batch B=512 across 8 cores (64 each).
  - Phase 1 (parallel): w_t = softmax(tanh(q@Wc.T+bc)@km.T) for ALL t (w is
    independent of the memory recurrence). Also precompute the a_t-dependent
    part of the erase/add gate logits. Stored to DRAM scratch.
  - Phase 2 (sequential over t): memory lives in SBUF as [128, 25*200] f16
    with partition p=(m%2)*64+b, free col=(m//2)*200+v.  Per step:
      wm = w*mem (ACT, per-partition scale), PE contracts wm with a constant
      block-diagonal ones matrix into PSUM -> rc0[b,v]; PE transposes rc0 and
      computes rc + gate logits (Wr folded into We/Wa on host); DVE applies
      mem = mem*(1-w*e) + w*ad via tensor ops.
  - Phase 3: final read at t=S-1, memory converted back to f32 and stored.
"""

import numpy as np

B, S, KD, M, V, A = 512, 1024, 50, 50, 200, 64
NC = 8
BL = B // NC  # 64 batches per core
NT = M // 2   # 25 m-pair tiles
MEMW = NT * V  # 5000


def build_bass(S_=S, BL_=BL, mem_f32=False):
    import concourse.bacc as bacc
    import concourse.bass as bass
    import concourse.tile as tile
    import concourse.mybir as mybir
    from contextlib import ExitStack

    f32 = mybir.dt.float32
    f16 = mybir.dt.float16
    AF = mybir.ActivationFunctionType
    OP = mybir.AluOpType
    AX = mybir.AxisListType

    nc = bacc.Bacc("TRN2", debug=False)

    # I/O
    q_d = nc.dram_tensor("q", [BL_, S_, KD], f32, kind="ExternalInput").ap()
    a_d = nc.dram_tensor("a", [BL_, S_, A], f32, kind="ExternalInput").ap()
    vm_d = nc.dram_tensor("vm", [BL_, M, V], f32, kind="ExternalInput").ap()
    ck_rhs_d = nc.dram_tensor("ck_rhs", [KD + 1, KD], f32, kind="ExternalInput").ap()
    kmT_d = nc.dram_tensor("kmT", [KD, M], f32, kind="ExternalInput").ap()
    WrT_d = nc.dram_tensor("WrT", [V, A], f32, kind="ExternalInput").ap()
    brr_d = nc.dram_tensor("brr", [1, A], f32, kind="ExternalInput").ap()
    Ge_d = nc.dram_tensor("Ge", [V, V], f32, kind="ExternalInput").ap()
    Ga_d = nc.dram_tensor("Ga", [V, V], f32, kind="ExternalInput").ap()
    ea_rhs_d = nc.dram_tensor("ea_rhs", [128, V], f32, kind="ExternalInput").ap()
    aa_rhs_d = nc.dram_tensor("aa_rhs", [128, V], f32, kind="ExternalInput").ap()
    ber_d = nc.dram_tensor("ber", [1, V], f32, kind="ExternalInput").ap()
    bar_d = nc.dram_tensor("bar", [1, V], f32, kind="ExternalInput").ap()
    on128_d = nc.dram_tensor("on128", [1, 128], f32, kind="ExternalInput").ap()
    obd_d = nc.dram_tensor("obd", [128, 64], f16, kind="ExternalInput").ap()
    i64_d = nc.dram_tensor("i64", [64, 64], f16, kind="ExternalInput").ap()
    idn_d = nc.dram_tensor("idn", [128, 128], f32, kind="ExternalInput").ap()
    on1_d = nc.dram_tensor("on1", [1, 64], f32, kind="ExternalInput").ap()

    rc_d = nc.dram_tensor("rc_out", [BL_, S_, A], f32, kind="ExternalOutput").ap()
    mf_d = nc.dram_tensor("mf_out", [BL_, M, V], f32, kind="ExternalOutput").ap()
    mw_d = nc.dram_tensor("mw_out", [BL_, S_, M], f32, kind="ExternalOutput").ap()

    wtr_d = nc.dram_tensor("wtr", [S_, 2, BL_, NT], f32, kind="Internal").ap()
    ea_d = nc.dram_tensor("ea_all", [S_, BL_, V], f16, kind="Internal").ap()
    aa_d = nc.dram_tensor("aa_all", [S_, BL_, V], f16, kind="Internal").ap()

    rc_r = rc_d.rearrange("b t a -> t b a")

    with tile.TileContext(nc) as tc, ExitStack() as ctx:
        cp = ctx.enter_context(tc.tile_pool(name="const", bufs=1))
        ck_rhs = cp.tile([KD + 1, KD], f32)
        kmT = cp.tile([KD, M], f32)
        WrT0 = cp.tile([100, A], f32)
        WrT1 = cp.tile([100, A], f32)
        brr = cp.tile([1, A], f32)
        Ge0 = cp.tile([100, V], f32)
        Ge1 = cp.tile([100, V], f32)
        Ga0 = cp.tile([100, V], f32)
        Ga1 = cp.tile([100, V], f32)
        ea_rhs = cp.tile([128, V], f32)
        aa_rhs = cp.tile([128, V], f32)
        ber = cp.tile([1, V], f32)
        bar = cp.tile([1, V], f32)
        on128 = cp.tile([1, 128], f32)
        obd = cp.tile([128, 64], f16)
        i64 = cp.tile([64, 64], f16)
        idn = cp.tile([128, 128], f32)
        on1 = cp.tile([1, 64], f32)
        nc.sync.dma_start(ck_rhs[:], ck_rhs_d)
        nc.sync.dma_start(kmT[:], kmT_d)
        nc.sync.dma_start(WrT0[:], WrT_d[0:100, :])
        nc.sync.dma_start(WrT1[:], WrT_d[100:200, :])
        nc.sync.dma_start(brr[:], brr_d)
        nc.sync.dma_start(Ge0[:], Ge_d[0:100, :])
        nc.sync.dma_start(Ge1[:], Ge_d[100:200, :])
        nc.sync.dma_start(Ga0[:], Ga_d[0:100, :])
        nc.sync.dma_start(Ga1[:], Ga_d[100:200, :])
        nc.sync.dma_start(ea_rhs[:], ea_rhs_d)
        nc.sync.dma_start(aa_rhs[:], aa_rhs_d)
        nc.sync.dma_start(ber[:], ber_d)
        nc.sync.dma_start(bar[:], bar_d)
        nc.sync.dma_start(on128[:], on128_d)
        nc.sync.dma_start(obd[:], obd_d)
        nc.sync.dma_start(i64[:], i64_d)
        nc.sync.dma_start(idn[:], idn_d)
        nc.sync.dma_start(on1[:], on1_d)

        mp = ctx.enter_context(tc.tile_pool(name="memp", bufs=1))
        mem = mp.tile([128, MEMW], f32 if mem_f32 else f16)
        wmt = mp.tile([128, MEMW], f16)
        t1t = mp.tile([128, MEMW], f16)
        stage = mp.tile([128, MEMW], f32)

        # ---- load value_memory into [128, 5000] layout, cast to f16 ----
        vm_r = vm_d.rearrange("b (j h) v -> h b j v", h=2)
        for h in range(2):
            nc.sync.dma_start(
                stage[h * 64:(h + 1) * 64, :].rearrange("b (j v) -> b j v", v=V),
                vm_r[h],
            )
        nc.vector.tensor_copy(mem[:], stage[:])

        # ---- phase 1 ----
        with (
            tc.tile_pool(name="p1", bufs=3) as p1,
            tc.tile_pool(name="p1s", bufs=2) as p1s,
            tc.tile_pool(name="p1ps", bufs=1, space="PSUM") as pp,
        ):
            n_tch = S_ // 128 if S_ >= 128 else 1
            tw = min(128, S_)
            for tch in range(n_tch):
                t0 = tch * tw

                def p1body(b):
                    qa = p1.tile([128, 128], f32, tag="qa")
                    if tw < 128:
                        nc.vector.memset(qa[:], 0.0)
                    nc.sync.dma_start(qa[0:tw, 0:KD], q_d[bass.ts(b, 1), t0:t0 + tw, :])
                    nc.vector.memset(qa[:, KD:64], 1.0)
                    nc.sync.dma_start(
                        qa[0:tw, 64:128], a_d[bass.ts(b, 1), t0:t0 + tw, :]
                    )
                    qaT_ps = pp.tile([128, 128], f32, tag="qaT_ps")
                    nc.tensor.transpose(qaT_ps[:], qa[:], idn[:])
                    qaT = p1.tile([128, 128], f32, tag="qaT")
                    nc.vector.tensor_copy(qaT[:], qaT_ps[:])
                    ck_ps = pp.tile([128, KD], f32, tag="ck_ps")
                    nc.tensor.matmul(ck_ps[:], qaT[0:KD + 1, :], ck_rhs[:], start=True, stop=True)
                    ckS = p1.tile([128, KD], f32, tag="ckS")
                    nc.scalar.activation(ckS[:], ck_ps[:], AF.Tanh)
                    ckT_ps = pp.tile([KD, 128], f32, tag="ckT_ps")
                    nc.tensor.transpose(ckT_ps[:], ckS[:], idn[:])
                    ckT = p1.tile([KD, 128], f32, tag="ckT")
                    nc.vector.tensor_copy(ckT[:], ckT_ps[:])
                    lg_ps = pp.tile([128, M], f32, tag="lg_ps")
                    nc.tensor.matmul(lg_ps[:], ckT[:], kmT[:], start=True, stop=True)
                    nmx = p1.tile([128, 1], f32, tag="nmx")
                    nc.vector.tensor_reduce(nmx[:], lg_ps[:], axis=AX.X, op=OP.max, negate=True)
                    ex = p1.tile([128, M], f32, tag="ex")
                    sm = p1.tile([128, 1], f32, tag="sm")
                    nc.scalar.activation(ex[:], lg_ps[:], AF.Exp, bias=nmx[:], accum_out=sm[:])
                    rs = p1.tile([128, 1], f32, tag="rs")
                    nc.vector.reciprocal(rs[:], sm[:])
                    wv = p1.tile([128, M], f32, tag="wv")
                    nc.vector.tensor_scalar(wv[:], ex[:], rs[:], None, op0=OP.mult)
                    nc.sync.dma_start(mw_d[bass.ts(b, 1), t0:t0 + tw, :], wv[0:tw, :])
                    wv2 = p1.tile([128, M], f32, tag="wv2")
                    nc.vector.tensor_copy(
                        wv2.rearrange("t (h j) -> t h j", h=2),
                        wv.rearrange("t (j h) -> t h j", h=2),
                    )
                    nc.sync.dma_start(
                        wtr_d[t0:t0 + tw, :, bass.ts(b, 1), :],
                        wv2[0:tw, :].rearrange("t (h j) -> t h j", h=2),
                    )
                    elg = pp.tile([128, V], f32, tag="elg")
                    nc.tensor.matmul(elg[:], qaT[64:128, :], ea_rhs[64:128, :], start=True, stop=False)
                    nc.tensor.matmul(elg[:], on128[:], ber[:], start=False, stop=True)
                    alg = pp.tile([128, V], f32, tag="alg")
                    nc.tensor.matmul(alg[:], qaT[64:128, :], aa_rhs[64:128, :], start=True, stop=False)
                    nc.tensor.matmul(alg[:], on128[:], bar[:], start=False, stop=True)
                    eaf = p1s.tile([128, V], f16, tag="eaf")
                    nc.scalar.activation(eaf[:], elg[:], AF.Copy)
                    aaf = p1s.tile([128, V], f16, tag="aaf")
                    nc.scalar.activation(aaf[:], alg[:], AF.Copy)
                    nc.sync.dma_start(ea_d[t0:t0 + tw, bass.ts(b, 1), :], eaf[0:tw, :])
                    nc.sync.dma_start(aa_d[t0:t0 + tw, bass.ts(b, 1), :], aaf[0:tw, :])

                tc.For_i_unrolled(0, BL_, 1, p1body, max_unroll=4)

        # ---- phase 2: sequential recurrence ----
        with (
            tc.tile_pool(name="p2", bufs=4) as p2,
            tc.tile_pool(name="p2b", bufs=2) as p2b,
            tc.tile_pool(name="ps1", bufs=1, space="PSUM") as ps1,
            tc.tile_pool(name="ps2", bufs=2, space="PSUM") as ps2,
            tc.tile_pool(name="ps3", bufs=1, space="PSUM") as ps3,
        ):
            def step(t, do_update):
                wsb = p2.tile([128, NT], f32, tag="wsb")
                nc.sync.dma_start(wsb[0:64, :], wtr_d[bass.ts(t, 1), 0])
                nc.sync.dma_start(wsb[64:128, :], wtr_d[bass.ts(t, 1), 1])
                if do_update:
                    easb = p2.tile([64, V], f16, tag="easb")
                    nc.sync.dma_start(easb[:], ea_d[bass.ts(t, 1)])
                    aasb = p2.tile([64, V], f16, tag="aasb")
                    nc.sync.dma_start(aasb[:], aa_d[bass.ts(t, 1)])
                # wm = w * mem  (ACT, per-partition scale)
                for j in range(NT):
                    js = slice(V * j, V * (j + 1))
                    nc.scalar.activation(wmt[:, js], mem[:, js], AF.Copy, scale=wsb[:, j:j + 1])
                # rc0[b,v] = sum_m w*mem : PE with constant block-diag ones
                rc0_ps = ps1.tile([64, V], f32, tag="rc0")
                for j in range(NT):
                    js = slice(V * j, V * (j + 1))
                    nc.tensor.matmul(
                        rc0_ps[:], obd[:], wmt[:, js], start=(j == 0), stop=(j == NT - 1)
                    )
                rc0S = p2b.tile([64, V], f32, tag="rc0S")
                nc.scalar.activation(rc0S[:], rc0_ps[:], AF.Copy)
                rcTa_ps = ps2.tile([100, 64], f32, tag="rcT_ps")
                nc.tensor.transpose(rcTa_ps[:], rc0S[:, 0:100], idn[0:64, 0:64])
                rcTb_ps = ps2.tile([100, 64], f32, tag="rcT_ps")
                nc.tensor.transpose(rcTb_ps[:], rc0S[:, 100:200], idn[0:64, 0:64])
                rcTa = p2b.tile([100, 64], f32, tag="rcTa")
                nc.scalar.activation(rcTa[:], rcTa_ps[:], AF.Copy)
                rcTb = p2b.tile([100, 64], f32, tag="rcTb")
                nc.scalar.activation(rcTb[:], rcTb_ps[:], AF.Copy)
                rc_ps = ps3.tile([64, A], f32, tag="rc_ps")
                nc.tensor.matmul(rc_ps[:], rcTa[:], WrT0[:], start=True, stop=False)
                nc.tensor.matmul(rc_ps[:], rcTb[:], WrT1[:], start=False, stop=False)
                nc.tensor.matmul(rc_ps[:], on1[:], brr[:], start=False, stop=True)
                rcS = p2.tile([64, A], f32, tag="rcS")
                nc.vector.tensor_copy(rcS[:], rc_ps[:])
                nc.sync.dma_start(rc_r[bass.ts(t, 1)], rcS[:])
                if not do_update:
                    return
                elog = ps1.tile([64, V], f32, tag="elog")
                nc.tensor.matmul(elog[:], rcTa[:], Ge0[:], start=True, stop=False)
                nc.tensor.matmul(elog[:], rcTb[:], Ge1[:], start=False, stop=False)
                nc.tensor.matmul(elog[:], i64[:], easb[:], start=False, stop=True)
                alog = ps1.tile([64, V], f32, tag="alog")
                nc.tensor.matmul(alog[:], rcTa[:], Ga0[:], start=True, stop=False)
                nc.tensor.matmul(alog[:], rcTb[:], Ga1[:], start=False, stop=False)
                nc.tensor.matmul(alog[:], i64[:], aasb[:], start=False, stop=True)
                edup = p2b.tile([128, V], f16, tag="edup")
                nc.scalar.activation(edup[0:64, :], elog[:], AF.Sigmoid)
                nc.scalar.activation(edup[64:128, :], elog[:], AF.Sigmoid)
                addup = p2b.tile([128, V], f16, tag="addup")
                nc.scalar.activation(addup[0:64, :], alog[:], AF.Tanh)
                nc.scalar.activation(addup[64:128, :], alog[:], AF.Tanh)
                # mem += (w*ad - e*(w*mem)) : single rounding of mem per step
                for j in range(NT):
                    js = slice(V * j, V * (j + 1))
                    nc.vector.tensor_mul(t1t[:, js], wmt[:, js], edup[:])
                for j in range(NT):
                    js = slice(V * j, V * (j + 1))
                    nc.vector.scalar_tensor_tensor(
                        t1t[:, js], addup[:], wsb[:, j:j + 1], t1t[:, js],
                        op0=OP.mult, op1=OP.subtract,
                    )
                nc.vector.tensor_add(mem[:], mem[:], t1t[:])

            tc.For_i_unrolled(0, S_ - 1, 1, lambda t: step(t, True), max_unroll=4)
            step(S_ - 1, False)

            nc.vector.tensor_copy(stage[:], mem[:])
            mf_r = mf_d.rearrange("b (j h) v -> h b j v", h=2)
            for h in range(2):
                nc.sync.dma_start(
                    mf_r[h],
                    stage[h * 64:(h + 1) * 64, :].rearrange("b (j v) -> b j v", v=V),
                )

    nc.compile()
    return nc


def _host_consts(key_memory, Wc, bc, Wr, br, We, be, Wa, ba):
    f32 = np.float32
    ck_rhs = np.concatenate([Wc.T, bc[None, :]], 0).astype(f32)        # [51, 50]
    kmT = key_memory.T.astype(f32)                                     # [50, 50]
    WrT = Wr.T.astype(f32)                                             # [200, 64]
    brr = br[None, :].astype(f32)                                      # [1, 64]
    WeR = We[:, A:].T                                                  # [64, 200]
    WaR = Wa[:, A:].T
    Ge = (Wr.T @ WeR).astype(f32)                                      # [200, 200]
    Ga = (Wr.T @ WaR).astype(f32)
    biasE = (br @ WeR + be).astype(f32)                                # [200]
    biasA = (br @ WaR + ba).astype(f32)
    ea_rhs = np.zeros((128, V), f32); ea_rhs[64:128] = We[:, :A].T
    aa_rhs = np.zeros((128, V), f32); aa_rhs[64:128] = Wa[:, :A].T
    ber = biasE[None, :]; bar = biasA[None, :]
    on128 = np.ones((1, 128), f32)
    obd = np.zeros((128, 64), np.float16)
    obd[np.arange(128), np.arange(128) % 64] = 1.0
    i64 = np.eye(64, dtype=np.float16)
    idn = np.eye(128, dtype=np.float32)
    on1 = np.ones((1, 64), np.float32)
    return dict(ck_rhs=ck_rhs, kmT=kmT, WrT=WrT, brr=brr, Ge=Ge, Ga=Ga,
                ea_rhs=ea_rhs, aa_rhs=aa_rhs, ber=ber, bar=bar, on128=on128,
                obd=obd, i64=i64, idn=idn, on1=on1)


_CACHED = {}


def kernel(query_embeddings, attention_features, value_memory, key_memory,
           Wc, bc, Wr, br, We, be, Wa, ba, _trace=False):
    from concourse.bass_utils import run_bass_kernel_spmd

    q = np.asarray(query_embeddings, np.float32)
    af = np.asarray(attention_features, np.float32)
    vm = np.asarray(value_memory, np.float32)
    consts = _host_consts(np.asarray(key_memory, np.float32), np.asarray(Wc, np.float32),
                          np.asarray(bc, np.float32), np.asarray(Wr, np.float32),
                          np.asarray(br, np.float32), np.asarray(We, np.float32),
                          np.asarray(be, np.float32), np.asarray(Wa, np.float32),
                          np.asarray(ba, np.float32))

    if "nc" not in _CACHED:
        _CACHED["nc"] = build_bass()
    nc = _CACHED["nc"]

    in_maps = []
    for c in range(NC):
        sl = slice(c * BL, (c + 1) * BL)
        m = {"q": np.ascontiguousarray(q[sl]),
             "a": np.ascontiguousarray(af[sl]),
             "vm": np.ascontiguousarray(vm[sl])}
        m.update(consts)
        in_maps.append(m)

    res = run_bass_kernel_spmd(nc, in_maps, core_ids=list(range(NC)), trace=_trace)
    rcs = np.concatenate([res.results[c]["rc_out"] for c in range(NC)], 0)
    mfs = np.concatenate([res.results[c]["mf_out"] for c in range(NC)], 0)
    mws = np.concatenate([res.results[c]["mw_out"] for c in range(NC)], 0)
    kernel._last_exec_ns = res.exec_time_ns
    return rcs, mfs, mws


# revision 9
# speedup vs baseline: 1.3674x; 1.3674x over previous
"""DKVMN attention-guided memory kernel for Trainium2, 8-core data-parallel.

Layout/algorithm:
  - Shard batch B=512 across 8 cores (64 each).
  - Phase 1 (parallel): w_t = softmax(tanh(q@Wc.T+bc)@km.T) for ALL t (w is
    independent of the memory recurrence). Also precompute the a_t-dependent
    part of the erase/add gate logits. Stored to DRAM scratch.
  - Phase 2 (sequential over t): memory lives in SBUF as [128, 25*200] f16
    with partition p=(m%2)*64+b, free col=(m//2)*200+v.  Per step:
      wm = w*mem (ACT, per-partition scale), PE contracts wm with a constant
      block-diagonal ones matrix into PSUM -> rc0[b,v]; PE transposes rc0 and
      computes rc + gate logits (Wr folded into We/Wa on host); DVE applies
      mem = mem*(1-w*e) + w*ad via tensor ops.
  - Phase 3: final read at t=S-1, memory converted back to f32 and stored.
"""

import numpy as np

B, S, KD, M, V, A = 512, 1024, 50, 50, 200, 64
NC = 8
BL = B // NC  # 64 batches per core
NT = M // 2   # 25 m-pair tiles
MEMW = NT * V  # 5000


def build_bass(S_=S, BL_=BL, mem_f32=False, n_steps=None, skip_p1=False):
    import concourse.bacc as bacc
    import concourse.bass as bass
    import concourse.tile as tile
    import concourse.mybir as mybir
    from contextlib import ExitStack

    f32 = mybir.dt.float32
    f16 = mybir.dt.float16
    AF = mybir.ActivationFunctionType
    OP = mybir.AluOpType
    AX = mybir.AxisListType

    nc = bacc.Bacc("TRN2", debug=False)

    # I/O
    q_d = nc.dram_tensor("q", [BL_, S_, KD], f32, kind="ExternalInput").ap()
    a_d = nc.dram_tensor("a", [BL_, S_, A], f32, kind="ExternalInput").ap()
    vm_d = nc.dram_tensor("vm", [BL_, M, V], f32, kind="ExternalInput").ap()
    ck_rhs_d = nc.dram_tensor("ck_rhs", [KD + 1, KD], f32, kind="ExternalInput").ap()
    kmT_d = nc.dram_tensor("kmT", [KD, M], f32, kind="ExternalInput").ap()
    WrT_d = nc.dram_tensor("WrT", [V, A], f32, kind="ExternalInput").ap()
    brr_d = nc.dram_tensor("brr", [1, A], f32, kind="ExternalInput").ap()
    Ge_d = nc.dram_tensor("Ge", [V, V], f32, kind="ExternalInput").ap()
    Ga_d = nc.dram_tensor("Ga", [V, V], f32, kind="ExternalInput").ap()
    ea_rhs_d = nc.dram_tensor("ea_rhs", [128, V], f32, kind="ExternalInput").ap()
    aa_rhs_d = nc.dram_tensor("aa_rhs", [128, V], f32, kind="ExternalInput").ap()
    ber_d = nc.dram_tensor("ber", [1, V], f32, kind="ExternalInput").ap()
    bar_d = nc.dram_tensor("bar", [1, V], f32, kind="ExternalInput").ap()
    on128_d = nc.dram_tensor("on128", [1, 128], f32, kind="ExternalInput").ap()
    obd_d = nc.dram_tensor("obd", [128, 64], f16, kind="ExternalInput").ap()
    i64_d = nc.dram_tensor("i64", [64, 64], f16, kind="ExternalInput").ap()
    idn_d = nc.dram_tensor("idn", [128, 128], f32, kind="ExternalInput").ap()
    on1_d = nc.dram_tensor("on1", [1, 64], f32, kind="ExternalInput").ap()

    rc_d = nc.dram_tensor("rc_out", [BL_, S_, A], f32, kind="ExternalOutput").ap()
    mf_d = nc.dram_tensor("mf_out", [BL_, M, V], f32, kind="ExternalOutput").ap()
    mw_d = nc.dram_tensor("mw_out", [BL_, S_, M], f32, kind="ExternalOutput").ap()

    wtr_d = nc.dram_tensor("wtr", [S_, 2, BL_, NT], f32, kind="Internal").ap()
    ea_d = nc.dram_tensor("ea_all", [S_, BL_, V], f16, kind="Internal").ap()
    aa_d = nc.dram_tensor("aa_all", [S_, BL_, V], f16, kind="Internal").ap()

    rc_r = rc_d.rearrange("b t a -> t b a")

    with tile.TileContext(nc) as tc, ExitStack() as ctx:
        cp = ctx.enter_context(tc.tile_pool(name="const", bufs=1))
        ck_rhs = cp.tile([KD + 1, KD], f32)
        kmT = cp.tile([KD, M], f32)
        WrT0 = cp.tile([100, A], f32)
        WrT1 = cp.tile([100, A], f32)
        brr = cp.tile([1, A], f32)
        Ge0 = cp.tile([100, V], f32)
        Ge1 = cp.tile([100, V], f32)
        Ga0 = cp.tile([100, V], f32)
        Ga1 = cp.tile([100, V], f32)
        ea_rhs = cp.tile([128, V], f32)
        aa_rhs = cp.tile([128, V], f32)
        ber = cp.tile([1, V], f32)
        bar = cp.tile([1, V], f32)
        on128 = cp.tile([1, 128], f32)
        obd = cp.tile([128, 64], f16)
        i64 = cp.tile([64, 64], f16)
        idn = cp.tile([128, 128], f32)
        on1 = cp.tile([1, 64], f32)
        nc.sync.dma_start(ck_rhs[:], ck_rhs_d)
        nc.sync.dma_start(kmT[:], kmT_d)
        nc.sync.dma_start(WrT0[:], WrT_d[0:100, :])
        nc.sync.dma_start(WrT1[:], WrT_d[100:200, :])
        nc.sync.dma_start(brr[:], brr_d)
        nc.sync.dma_start(Ge0[:], Ge_d[0:100, :])
        nc.sync.dma_start(Ge1[:], Ge_d[100:200, :])
        nc.sync.dma_start(Ga0[:], Ga_d[0:100, :])
        nc.sync.dma_start(Ga1[:], Ga_d[100:200, :])
        nc.sync.dma_start(ea_rhs[:], ea_rhs_d)
        nc.sync.dma_start(aa_rhs[:], aa_rhs_d)
        nc.sync.dma_start(ber[:], ber_d)
        nc.sync.dma_start(bar[:], bar_d)
        nc.sync.dma_start(on128[:], on128_d)
        nc.sync.dma_start(obd[:], obd_d)
        nc.sync.dma_start(i64[:], i64_d)
        nc.sync.dma_start(idn[:], idn_d)
        nc.sync.dma_start(on1[:], on1_d)

        mp = ctx.enter_context(tc.tile_pool(name="memp", bufs=1))
        mem = mp.tile([128, MEMW], f32 if mem_f32 else f16)
        wmt = mp.tile([128, MEMW], f16)
        t1t = mp.tile([128, MEMW], f16)
        stage = mp.tile([128, MEMW], f32)

        # ---- load value_memory into [128, 5000] layout, cast to f16 ----
        vm_r = vm_d.rearrange("b (j h) v -> h b j v", h=2)
        for h in range(2):
            nc.sync.dma_start(
                stage[h * 64:(h + 1) * 64, :].rearrange("b (j v) -> b j v", v=V),
                vm_r[h],
            )
        nc.vector.tensor_copy(mem[:], stage[:])

        # ---- phase 1 ----
        with (
            tc.tile_pool(name="p1", bufs=3) as p1,
            tc.tile_pool(name="p1s", bufs=2) as p1s,
            tc.tile_pool(name="p1ps", bufs=1, space="PSUM") as pp,
        ):
            n_tch = S_ // 128 if S_ >= 128 else 1
            tw = min(128, S_)
            for tch in range(n_tch):
                t0 = tch * tw

                def p1body(b):
                    qa = p1.tile([128, 128], f32, tag="qa")
                    if tw < 128:
                        nc.vector.memset(qa[:], 0.0)
                    nc.sync.dma_start(qa[0:tw, 0:KD], q_d[bass.ts(b, 1), t0:t0 + tw, :])
                    nc.vector.memset(qa[:, KD:64], 1.0)
                    nc.sync.dma_start(
                        qa[0:tw, 64:128], a_d[bass.ts(b, 1), t0:t0 + tw, :]
                    )
                    qaT_ps = pp.tile([128, 128], f32, tag="qaT_ps")
                    nc.tensor.transpose(qaT_ps[:], qa[:], idn[:])
                    qaT = p1.tile([128, 128], f32, tag="qaT")
                    nc.vector.tensor_copy(qaT[:], qaT_ps[:])
                    ck_ps = pp.tile([128, KD], f32, tag="ck_ps")
                    nc.tensor.matmul(ck_ps[:], qaT[0:KD + 1, :], ck_rhs[:], start=True, stop=True)
                    ckS = p1.tile([128, KD], f32, tag="ckS")
                    nc.scalar.activation(ckS[:], ck_ps[:], AF.Tanh)
                    ckT_ps = pp.tile([KD, 128], f32, tag="ckT_ps")
                    nc.tensor.transpose(ckT_ps[:], ckS[:], idn[:])
                    ckT = p1.tile([KD, 128], f32, tag="ckT")
                    nc.vector.tensor_copy(ckT[:], ckT_ps[:])
                    lg_ps = pp.tile([128, M], f32, tag="lg_ps")
                    nc.tensor.matmul(lg_ps[:], ckT[:], kmT[:], start=True, stop=True)
                    nmx = p1.tile([128, 1], f32, tag="nmx")
                    nc.vector.tensor_reduce(nmx[:], lg_ps[:], axis=AX.X, op=OP.max, negate=True)
                    ex = p1.tile([128, M], f32, tag="ex")
                    sm = p1.tile([128, 1], f32, tag="sm")
                    nc.scalar.activation(ex[:], lg_ps[:], AF.Exp, bias=nmx[:], accum_out=sm[:])
                    rs = p1.tile([128, 1], f32, tag="rs")
                    nc.vector.reciprocal(rs[:], sm[:])
                    wv = p1.tile([128, M], f32, tag="wv")
                    nc.vector.tensor_scalar(wv[:], ex[:], rs[:], None, op0=OP.mult)
                    nc.sync.dma_start(mw_d[bass.ts(b, 1), t0:t0 + tw, :], wv[0:tw, :])
                    wv2 = p1.tile([128, M], f32, tag="wv2")
                    nc.vector.tensor_copy(
                        wv2.rearrange("t (h j) -> t h j", h=2),
                        wv.rearrange("t (j h) -> t h j", h=2),
                    )
                    nc.sync.dma_start(
                        wtr_d[t0:t0 + tw, :, bass.ts(b, 1), :],
                        wv2[0:tw, :].rearrange("t (h j) -> t h j", h=2),
                    )
                    elg = pp.tile([128, V], f32, tag="elg")
                    nc.tensor.matmul(elg[:], qaT[64:128, :], ea_rhs[64:128, :], start=True, stop=False)
                    nc.tensor.matmul(elg[:], on128[:], ber[:], start=False, stop=True)
                    alg = pp.tile([128, V], f32, tag="alg")
                    nc.tensor.matmul(alg[:], qaT[64:128, :], aa_rhs[64:128, :], start=True, stop=False)
                    nc.tensor.matmul(alg[:], on128[:], bar[:], start=False, stop=True)
                    eaf = p1s.tile([128, V], f16, tag="eaf")
                    nc.scalar.activation(eaf[:], elg[:], AF.Copy)
                    aaf = p1s.tile([128, V], f16, tag="aaf")
                    nc.scalar.activation(aaf[:], alg[:], AF.Copy)
                    nc.sync.dma_start(ea_d[t0:t0 + tw, bass.ts(b, 1), :], eaf[0:tw, :])
                    nc.sync.dma_start(aa_d[t0:t0 + tw, bass.ts(b, 1), :], aaf[0:tw, :])

                tc.For_i_unrolled(0, 0 if skip_p1 else BL_, 1, p1body, max_unroll=4)

        # ---- phase 2: sequential recurrence ----
        with (
            tc.tile_pool(name="p2", bufs=4) as p2,
            tc.tile_pool(name="p2b", bufs=2) as p2b,
            tc.tile_pool(name="ps1", bufs=1, space="PSUM") as ps1,
            tc.tile_pool(name="ps2", bufs=2, space="PSUM") as ps2,
            tc.tile_pool(name="ps3", bufs=1, space="PSUM") as ps3,
        ):
            def step(t, do_update):
                wsb = p2.tile([128, NT], f32, tag="wsb")
                nc.sync.dma_start(wsb[0:64, :], wtr_d[bass.ts(t, 1), 0])
                nc.sync.dma_start(wsb[64:128, :], wtr_d[bass.ts(t, 1), 1])
                if do_update:
                    easb = p2.tile([64, V], f16, tag="easb")
                    nc.sync.dma_start(easb[:], ea_d[bass.ts(t, 1)])
                    aasb = p2.tile([64, V], f16, tag="aasb")
                    nc.sync.dma_start(aasb[:], aa_d[bass.ts(t, 1)])
                # wm = w * mem  (ACT, per-partition scale)
                for j in range(NT):
                    js = slice(V * j, V * (j + 1))
                    nc.scalar.activation(wmt[:, js], mem[:, js], AF.Copy, scale=wsb[:, j:j + 1])
                # rc0[b,v] = sum_m w*mem : PE with constant block-diag ones
                rc0_ps = ps1.tile([64, V], f32, tag="rc0")
                for j in range(NT):
                    js = slice(V * j, V * (j + 1))
                    nc.tensor.matmul(
                        rc0_ps[:], obd[:], wmt[:, js], start=(j == 0), stop=(j == NT - 1)
                    )
                rc0S = p2b.tile([64, V], f32, tag="rc0S")
                nc.scalar.activation(rc0S[:], rc0_ps[:], AF.Copy)
                rcTa_ps = ps2.tile([100, 64], f32, tag="rcT_ps")
                nc.tensor.transpose(rcTa_ps[:], rc0S[:, 0:100], idn[0:64, 0:64])
                rcTb_ps = ps2.tile([100, 64], f32, tag="rcT_ps")
                nc.tensor.transpose(rcTb_ps[:], rc0S[:, 100:200], idn[0:64, 0:64])
                rcTa = p2b.tile([100, 64], f32, tag="rcTa")
                nc.scalar.activation(rcTa[:], rcTa_ps[:], AF.Copy)
                rcTb = p2b.tile([100, 64], f32, tag="rcTb")
                nc.scalar.activation(rcTb[:], rcTb_ps[:], AF.Copy)
                rc_ps = ps3.tile([64, A], f32, tag="rc_ps")
                nc.tensor.matmul(rc_ps[:], rcTa[:], WrT0[:], start=True, stop=False)
                nc.tensor.matmul(rc_ps[:], rcTb[:], WrT1[:], start=False, stop=False)
                nc.tensor.matmul(rc_ps[:], on1[:], brr[:], start=False, stop=True)
                rcS = p2.tile([64, A], f32, tag="rcS")
                nc.vector.tensor_copy(rcS[:], rc_ps[:])
                nc.sync.dma_start(rc_r[bass.ts(t, 1)], rcS[:])
                if not do_update:
                    return
                elog = ps1.tile([64, V], f32, tag="elog")
                nc.tensor.matmul(elog[:], rcTa[:], Ge0[:], start=True, stop=False)
                nc.tensor.matmul(elog[:], rcTb[:], Ge1[:], start=False, stop=False)
                nc.tensor.matmul(elog[:], i64[:], easb[:], start=False, stop=True)
                alog = ps1.tile([64, V], f32, tag="alog")
                nc.tensor.matmul(alog[:], rcTa[:], Ga0[:], start=True, stop=False)
                nc.tensor.matmul(alog[:], rcTb[:], Ga1[:], start=False, stop=False)
                nc.tensor.matmul(alog[:], i64[:], aasb[:], start=False, stop=True)
                edup = p2b.tile([128, V], f16, tag="edup")
                nc.scalar.activation(edup[0:64, :], elog[:], AF.Sigmoid)
                nc.scalar.activation(edup[64:128, :], elog[:], AF.Sigmoid)
                addup = p2b.tile([128, V], f16, tag="addup")
                nc.scalar.activation(addup[0:64, :], alog[:], AF.Tanh)
                nc.scalar.activation(addup[64:128, :], alog[:], AF.Tanh)
                # mem += (w*ad - e*(w*mem)) : single rounding of mem per step
                for j in range(NT):
                    js = slice(V * j, V * (j + 1))
                    nc.vector.tensor_mul(t1t[:, js], wmt[:, js], edup[:])
                for j in range(NT):
                    js = slice(V * j, V * (j + 1))
                    nc.vector.scalar_tensor_tensor(
                        t1t[:, js], addup[:], wsb[:, j:j + 1], t1t[:, js],
                        op0=OP.mult, op1=OP.subtract,
                    )
                nc.vector.tensor_add(mem[:], mem[:], t1t[:])

            tc.For_i_unrolled(0, (S_ - 1) if n_steps is None else n_steps, 1, lambda t: step(t, True), max_unroll=4)
            step(S_ - 1, False)

            nc.vector.tensor_copy(stage[:], mem[:])
            mf_r = mf_d.rearrange("b (j h) v -> h b j v", h=2)
            for h in range(2):
                nc.sync.dma_start(
                    mf_r[h],
                    stage[h * 64:(h + 1) * 64, :].rearrange("b (j v) -> b j v", v=V),
                )

    nc.compile()
    return nc


def _host_consts(key_memory, Wc, bc, Wr, br, We, be, Wa, ba):
    f32 = np.float32
    ck_rhs = np.concatenate([Wc.T, bc[None, :]], 0).astype(f32)        # [51, 50]
    kmT = key_memory.T.astype(f32)                                     # [50, 50]
    WrT = Wr.T.astype(f32)                                             # [200, 64]
    brr = br[None, :].astype(f32)                                      # [1, 64]
    WeR = We[:, A:].T                                                  # [64, 200]
    WaR = Wa[:, A:].T
    Ge = (Wr.T @ WeR).astype(f32)                                      # [200, 200]
    Ga = (Wr.T @ WaR).astype(f32)
    biasE = (br @ WeR + be).astype(f32)                                # [200]
    biasA = (br @ WaR + ba).astype(f32)
    ea_rhs = np.zeros((128, V), f32); ea_rhs[64:128] = We[:, :A].T
    aa_rhs = np.zeros((128, V), f32); aa_rhs[64:128] = Wa[:, :A].T
    ber = biasE[None, :]; bar = biasA[None, :]
    on128 = np.ones((1, 128), f32)
    obd = np.zeros((128, 64), np.float16)
    obd[np.arange(128), np.arange(128) % 64] = 1.0
    i64 = np.eye(64, dtype=np.float16)
    idn = np.eye(128, dtype=np.float32)
    on1 = np.ones((1, 64), np.float32)
    return dict(ck_rhs=ck_rhs, kmT=kmT, WrT=WrT, brr=brr, Ge=Ge, Ga=Ga,
                ea_rhs=ea_rhs, aa_rhs=aa_rhs, ber=ber, bar=bar, on128=on128,
                obd=obd, i64=i64, idn=idn, on1=on1)


_CACHED = {}


def kernel(query_embeddings, attention_features, value_memory, key_memory,
           Wc, bc, Wr, br, We, be, Wa, ba, _trace=False):
    from concourse.bass_utils import run_bass_kernel_spmd

    q = np.asarray(query_embeddings, np.float32)
    af = np.asarray(attention_features, np.float32)
    vm = np.asarray(value_memory, np.float32)
    consts = _host_consts(np.asarray(key_memory, np.float32), np.asarray(Wc, np.float32),
                          np.asarray(bc, np.float32), np.asarray(Wr, np.float32),
                          np.asarray(br, np.float32), np.asarray(We, np.float32),
                          np.asarray(be, np.float32), np.asarray(Wa, np.float32),
                          np.asarray(ba, np.float32))

    if "nc" not in _CACHED:
        _CACHED["nc"] = build_bass()
    nc = _CACHED["nc"]

    in_maps = []
    for c in range(NC):
        sl = slice(c * BL, (c + 1) * BL)
        m = {"q": np.ascontiguousarray(q[sl]),
             "a": np.ascontiguousarray(af[sl]),
             "vm": np.ascontiguousarray(vm[sl])}
        m.update(consts)
        in_maps.append(m)

    res = run_bass_kernel_spmd(nc, in_maps, core_ids=list(range(NC)), trace=_trace)
    rcs = np.concatenate([res.results[c]["rc_out"] for c in range(NC)], 0)
    mfs = np.concatenate([res.results[c]["mf_out"] for c in range(NC)], 0)
    mws = np.concatenate([res.results[c]["mw_out"] for c in range(NC)], 0)
    kernel._last_exec_ns = res.exec_time_ns
    return rcs, mfs, mws
